# revision 1
# baseline (speedup 1.0000x reference)
"""Trainium2 Bass kernel for nn_Model1_52518860096440 (dense_transformer).

Reference computation (B=4, S=4096, HID=1024, H=16, DH=64):
    qkv = query @ W_qkv.T + b_qkv            # only `query` is used
    q, k, v = split(qkv); reshape to (B,S,H,DH)
    s = einsum('bshd,bsgd->bshg', q, k) / 8 + attn_mask   # per-position head mixing
    p = softmax(s, -1)
    out = einsum('bshg,bsgd->bshd', p, v).reshape(B,S,HID)

Strategy: shard the B*S = 16384 tokens across 8 cores (2048 each).
W_qkv is replicated. Per core:
  - Phase 1: QKV projection as fp32r matmuls (full PE rate at N=512),
    lhsT = query^T tiles (host-transposed), rhs = W^T tiles (host-transposed,
    attention scale 1/8 pre-folded into the q columns of W and b).
  - Phase 2: per-position 16x16 attention via fused vector ops:
    scores: per k-head g, tensor_mul (q-part x k_g broadcast) + tensor_reduce
    softmax: mask add + exp (ACT) + strided reduce + reciprocal
    AV: scalar_tensor_tensor accumulation chains, h-split across DVE/GPSIMD.
"""

from contextlib import ExitStack

import numpy as np

B, S, HID, H = 4, 4096, 1024, 16
DH = HID // H                 # 64
NCORES = 8
T = B * S                     # 16384 tokens
TC = T // NCORES              # 2048 tokens per core
P = 128                       # partitions / tokens per tile
NT = TC // P                  # 16 token tiles per core
KT = HID // P                 # 8 contraction tiles
OC = 512                      # output-chunk for QKV matmuls
NOC = 3 * HID // OC           # 6 chunks
H_DVE = 10                    # h-heads handled on DVE (rest on GPSIMD)

_compiled = {}


def _build(phase=4, sc_gps=14, av_gps=12):
    import concourse.bass as bass
    import concourse.tile as tile
    import concourse.mybir as mybir
    from concourse import bacc

    f32 = mybir.dt.float32
    f16 = mybir.dt.float16
    f32r = mybir.dt.float32r
    Alu = mybir.AluOpType
    Act = mybir.ActivationFunctionType

    nc = bacc.Bacc("TRN2", target_bir_lowering=False, debug=False,
                   num_devices=NCORES)

    xT_d = nc.dram_tensor("xT", (HID, TC), f32r, kind="ExternalInput")
    wT_d = nc.dram_tensor("wT", (HID, 3 * HID), f32r, kind="ExternalInput")
    bias_d = nc.dram_tensor("biasbc", (P, 3 * HID), f32, kind="ExternalInput")
    mask_d = nc.dram_tensor("maskp", (TC, H * H), f32, kind="ExternalInput")
    out_d = nc.dram_tensor("out", (TC, HID), f32, kind="ExternalOutput")

    with tile.TileContext(nc) as tc, ExitStack() as ctx:
        const = ctx.enter_context(tc.tile_pool(name="const", bufs=1))
        xpool = ctx.enter_context(tc.tile_pool(name="x", bufs=3))
        qkvp = ctx.enter_context(tc.tile_pool(name="qkv", bufs=3))
        work = ctx.enter_context(tc.tile_pool(name="work", bufs=4))
        opool = ctx.enter_context(tc.tile_pool(name="o", bufs=2))
        psum = ctx.enter_context(tc.tile_pool(name="ps", bufs=3, space="PSUM"))

        # ---- resident weights / bias ----
        w_tiles = []
        wT_r = wT_d[:].rearrange("(kt kp) o -> kp kt o", kp=P)
        for kt in range(KT):
            row = []
            for oc in range(NOC):
                wt = const.tile([P, OC], f32r, tag=f"w{kt}_{oc}")
                nc.sync.dma_start(wt[:], wT_r[:, kt, oc * OC:(oc + 1) * OC])
                row.append(wt)
            w_tiles.append(row)
        bias_t = const.tile([P, 3 * HID], f32)
        nc.sync.dma_start(bias_t[:], bias_d[:])
        neg4 = const.tile([P, 1], f32, tag="neg4")
        nc.vector.memset(neg4[:], -4.0)
        ones_r = const.tile([1, P], f32, tag="ones_r")
        nc.vector.memset(ones_r[:], 1.0)

        xT_r = xT_d[:].rearrange("(kt kp) t -> kp kt t", kp=P)

        for tt in range(NT):
            tsl = slice(tt * P, (tt + 1) * P)

            # ---- phase 1: QKV = x @ W^T + b ----
            x_tiles = []
            for kt in range(KT):
                xk = xpool.tile([P, P], f32r, tag=f"x{kt}")
                nc.sync.dma_start(xk[:], xT_r[:, kt, tsl])
                x_tiles.append(xk)

            qkv = qkvp.tile([P, 3 * HID], f16, tag="qkv")
            for oc in range(NOC):
                acc = psum.tile([P, OC], f32, tag="acc")
                osl = slice(oc * OC, (oc + 1) * OC)
                for kt in range(KT):
                    nc.tensor.matmul(acc[:], x_tiles[kt][:],
                                     w_tiles[kt][oc][:],
                                     start=(kt == 0), stop=False)
                # bias as a K=1 ones-row matmul accumulated into PSUM
                nc.tensor.matmul(acc[:], ones_r[:], bias_t[0:1, osl],
                                 start=False, stop=True)
                # psum -> sbuf copy on ACT (frees DVE)
                nc.scalar.copy(qkv[:, osl], acc[:])

            if phase <= 1:
                nc.sync.dma_start(out_d[tsl, :], qkv[:, 0:HID])
                continue

            qp = qkv[:, 0:HID].rearrange("p (h d) -> p h d", d=DH)

            # ---- phase 2a: scores s[t, g*16+h] = sum_d q[t,h,d] k[t,g,d] ----
            s_t = work.tile([P, H * H], f32, tag="s")
            for g in range(H):
                kg = qkv[:, HID + g * DH: HID + (g + 1) * DH]
                kg_b = kg.unsqueeze(1).broadcast_to((P, H, DH))
                tmp = work.tile([P, H, DH], f16, tag=f"tmp{g % 2}")
                mul_eng = nc.gpsimd if g < sc_gps else nc.vector
                mul_eng.tensor_mul(tmp[:], qp, kg_b)
                nc.vector.tensor_reduce(
                    s_t[:, g * H:(g + 1) * H], tmp[:],
                    axis=mybir.AxisListType.X, op=Alu.add)

            if phase <= 2:
                nc.sync.dma_start(out_d[tsl, 0:H * H], s_t[:])
                continue

            # ---- phase 2b: softmax (no max-sub; logits are O(10)) ----
            m_t = work.tile([P, H * H], f32, tag="m")
            nc.sync.dma_start(m_t[:], mask_d[tsl, :])
            sm_t = work.tile([P, H * H], f32, tag="sm")
            nc.vector.tensor_add(sm_t[:], s_t[:], m_t[:])
            e_t = work.tile([P, H * H], f16, tag="e")
            # exp(x - 4): constant shift cancels in softmax, keeps f16 finite
            nc.scalar.activation(e_t[:], sm_t[:], Act.Exp, bias=neg4[:])
            sums = work.tile([P, H], f32, tag="sums")
            nc.vector.tensor_reduce(
                sums[:], e_t[:].rearrange("p (g h) -> p h g", g=H),
                axis=mybir.AxisListType.X, op=Alu.add)
            recip = work.tile([P, H], f32, tag="recip")
            nc.vector.reciprocal(recip[:], sums[:])

            if phase <= 3:
                nc.sync.dma_start(out_d[tsl, 0:H * H], e_t[:])
                continue

            # ---- phase 2c: AV = sum_g p[t,h,g] v[t,g,:] ----
            # per h: gpsimd broadcast-mult over (g,d), DVE strided reduce over g
            vpart = qkv[:, 2 * HID:3 * HID].rearrange("p (g d) -> p g d", d=DH)
            o_t = opool.tile([P, HID], f32, tag="out")
            e3 = e_t[:].rearrange("p (g h) -> p g h", g=H)
            for h in range(H):
                # eh_b[t, g, d] = e[t, g*16+h]  (broadcast over d)
                eh_b = e3[:, :, h].unsqueeze(2).broadcast_to((P, H, DH))
                tmp = work.tile([P, H, DH], f16, tag=f"avt{h % 2}")
                mul_eng = nc.gpsimd if h < av_gps else nc.vector
                mul_eng.tensor_mul(tmp[:], vpart, eh_b)
                # reduce over g: view tmp as (p, d, g) via strides
                nc.vector.tensor_reduce(
                    o_t[:, h * DH:(h + 1) * DH],
                    tmp[:].transpose([0, 2, 1]),
                    axis=mybir.AxisListType.X, op=Alu.add)

            # ---- normalize and store ----
            r_b = recip[:].unsqueeze(2).broadcast_to((P, H, DH))
            of = opool.tile([P, HID], f32, tag="outf")
            nc.vector.tensor_mul(
                of[:].rearrange("p (h d) -> p h d", d=DH),
                o_t[:].rearrange("p (h d) -> p h d", d=DH), r_b)
            nc.sync.dma_start(out_d[tsl, :], of[:])

    nc.compile()
    return nc


def _host_prep(query, W_qkv, b_qkv, attn_mask):
    x = np.ascontiguousarray(query.reshape(T, HID), dtype=np.float32)
    xT = np.ascontiguousarray(x.T)                       # (HID, T)
    wT = np.ascontiguousarray(W_qkv.T, dtype=np.float32)  # (HID, 3*HID)
    b = np.array(b_qkv, dtype=np.float32).copy()
    scale = 1.0 / np.sqrt(DH)
    wT[:, 0:HID] *= scale
    b[0:HID] *= scale
    bias_bc = np.ascontiguousarray(np.broadcast_to(b, (P, 3 * HID)))
    # mask packed as [t, g*16+h] = attn_mask[t, h, g]
    m = np.asarray(attn_mask, dtype=np.float32).reshape(T, H, H)
    maskp = np.ascontiguousarray(m.transpose(0, 2, 1).reshape(T, H * H))
    return xT, wT, bias_bc, maskp


def kernel(query, key, value, attn_mask, W_qkv, b_qkv):
    from concourse.bass_utils import run_bass_kernel_spmd

    xT, wT, bias_bc, maskp = _host_prep(query, W_qkv, b_qkv, attn_mask)

    if "nc" not in _compiled:
        _compiled["nc"] = _build()
    nc = _compiled["nc"]

    in_maps = []
    for c in range(NCORES):
        tsl = slice(c * TC, (c + 1) * TC)
        in_maps.append({
            "xT": np.ascontiguousarray(xT[:, tsl]),
            "wT": wT,
            "biasbc": bias_bc,
            "maskp": np.ascontiguousarray(maskp[tsl, :]),
        })

    res = run_bass_kernel_spmd(nc, in_maps, core_ids=list(range(NCORES)))
    out = np.concatenate([r["out"] for r in res.results], axis=0)
    return out.reshape(B, S, HID).astype(np.float32)


if __name__ == "__main__":
    rng = np.random.default_rng(0)
    inputs = {
        "query": rng.standard_normal((B, S, HID), dtype=np.float32),
        "key": rng.standard_normal((B, S, HID), dtype=np.float32),
        "value": rng.standard_normal((B, S, HID), dtype=np.float32),
        "attn_mask": rng.standard_normal((B, S, H, H), dtype=np.float32),
        "W_qkv": (rng.standard_normal((3 * HID, HID), dtype=np.float32)
                  / np.sqrt(HID)),
        "b_qkv": rng.standard_normal((3 * HID,), dtype=np.float32) * 0.01,
    }
    out = kernel(**inputs)
    print("kernel output:", out.shape, out.dtype, np.abs(out).mean())



# revision 7
# speedup vs baseline: 3.9094x; 3.9094x over previous
"""Trainium2 Bass kernel for nn_Model1_52518860096440 (dense_transformer).

Reference (B=4, S=4096, HID=1024, H=16, DH=64):
    qkv = query @ W_qkv.T + b_qkv           # only `query` is used
    q, k, v = split(qkv) -> (B,S,H,DH)
    s = einsum('bshd,bsgd->bshg', q, k) / 8 + attn_mask   # per-position 16x16
    p = softmax(s, -1); out = einsum('bshg,bsgd->bshd', p, v)

Strategy: 16384 tokens sharded 8 ways (2048/core, 16 tiles of 128).
Phase 2 runs almost entirely on the PE via per-token 16-row matmuls:
  - projection produces q/k in [d, h, t] layout (per-head columns), v
    token-major; v round-trips DRAM into [g, slot, d] replicas at
    partition offsets 32*(j%4).
  - scores: per-token matmul k_t[d,g]^T q_t[d,h] parked in PSUM at
    [32*(j%4)+g, 16*(j//4)+h]; softmax denominators via one static
    block-ones matmul (sums replicated); exp on ACT; normalize on DVE.
  - AV: per-token matmul vrep[g,d]^T e2s[g,h] parked at
    [64*(j%2)+d, 16*(j//2)+h]; raw parked layout is dumped and the host
    decodes it.
"""

from contextlib import ExitStack

import numpy as np

B, S, HID, H = 4, 4096, 1024, 16
DH = HID // H                 # 64
NCORES = 8
T = B * S                     # 16384 tokens
TC = T // NCORES              # 2048 tokens per core
P = 128
NT = TC // P                  # 16 token tiles per core
KT = HID // P                 # 8 contraction chunks
NEG = -30000.0                # mask fill for dead partition rows

_compiled = {}


def _build(phase=4):
    import concourse.bass as bass
    import concourse.tile as tile
    import concourse.mybir as mybir
    from concourse import bacc

    f32 = mybir.dt.float32
    f16 = mybir.dt.float16
    Alu = mybir.AluOpType
    Act = mybir.ActivationFunctionType

    nc = bacc.Bacc("TRN2", target_bir_lowering=False, debug=False,
                   num_devices=NCORES)

    xk_d = nc.dram_tensor("xk", (NT, P, KT, P), f16, kind="ExternalInput")
    wqk_d = nc.dram_tensor("wqk", (P, H, KT, P), f16, kind="ExternalInput")
    bqk_d = nc.dram_tensor("bqk", (1, H, P), f16, kind="ExternalInput")
    wv_d = nc.dram_tensor("wv", (P, KT, HID), f16, kind="ExternalInput")
    bv_d = nc.dram_tensor("bv", (1, HID), f16, kind="ExternalInput")
    m2_d = nc.dram_tensor("m2", (NT, P, 512), f16, kind="ExternalInput")
    obd_d = nc.dram_tensor("obd", (P, P), f16, kind="ExternalInput")
    out_d = nc.dram_tensor("out", (NT, P, HID), f16, kind="ExternalOutput")

    with tile.TileContext(nc) as tc, ExitStack() as ctx:
        const = ctx.enter_context(tc.tile_pool(name="const", bufs=1))
        xpool = ctx.enter_context(tc.tile_pool(name="x", bufs=2))
        qkpool = ctx.enter_context(tc.tile_pool(name="qk", bufs=2))
        vpool = ctx.enter_context(tc.tile_pool(name="v", bufs=2))
        vreppool = ctx.enter_context(tc.tile_pool(name="vrep", bufs=3))
        mpool = ctx.enter_context(tc.tile_pool(name="m", bufs=3))
        epool = ctx.enter_context(tc.tile_pool(name="e", bufs=2))
        opool = ctx.enter_context(tc.tile_pool(name="o", bufs=2))
        dpool = ctx.enter_context(tc.tile_pool(name="dscr", bufs=2,
                                               space="DRAM"))
        psq = ctx.enter_context(tc.tile_pool(name="psq", bufs=2, space="PSUM"))
        psv = ctx.enter_context(tc.tile_pool(name="psv", bufs=2, space="PSUM"))
        pss = ctx.enter_context(tc.tile_pool(name="pss", bufs=1, space="PSUM"))
        pssum = ctx.enter_context(tc.tile_pool(name="pssum", bufs=1,
                                               space="PSUM"))
        psav = ctx.enter_context(tc.tile_pool(name="psav", bufs=1,
                                              space="PSUM"))

        # ---- resident constants ----
        wqk_sb = const.tile([P, H, KT, P], f16)
        nc.sync.dma_start(wqk_sb[:], wqk_d[:])
        bqk_sb = const.tile([1, H, P], f16)
        nc.sync.dma_start(bqk_sb[:], bqk_d[:])
        wv_sb = const.tile([P, KT, HID], f16)
        nc.sync.dma_start(wv_sb[:], wv_d[:])
        bv_sb = const.tile([1, HID], f16)
        nc.sync.dma_start(bv_sb[:], bv_d[:])
        obd_sb = const.tile([P, P], f16)
        nc.sync.dma_start(obd_sb[:], obd_d[:])
        ones_row = const.tile([1, P], f16, tag="ones_row")
        nc.vector.memset(ones_row[:], 1.0)
        neg2 = const.tile([P, 1], f32, tag="neg2")
        nc.vector.memset(neg2[:], -2.0)

        # persistent scores psum bank; dead rows zeroed once
        sps = pss.tile([P, 512], f32)
        nc.vector.memset(sps[:], 0.0)

        # per-iteration state carried between pipeline stages
        st = {}

        def stage_a(t):
            """Projection for tile t: q/k per-head layout + v roundtrip."""
            xk = xpool.tile([P, KT, P], f16, tag="xk")
            nc.sync.dma_start(xk[:], xk_d[t])
            m2 = mpool.tile([P, 512], f16, tag="m2")
            nc.sync.dma_start(m2[:], m2_d[t])

            q_sb = qkpool.tile([64, H, P], f16, tag="q")
            k_sb = qkpool.tile([64, H, P], f16, tag="k")
            for hg in range(4):
                ps = psq.tile([P, 512], f32, tag="qkps")
                for hh in range(4):
                    h = hg * 4 + hh
                    osl = slice(hh * P, (hh + 1) * P)
                    for kt in range(KT):
                        nc.tensor.matmul(ps[:, osl], wqk_sb[:, h, kt, :],
                                         xk[:, kt, :],
                                         start=(kt == 0), stop=False)
                    nc.tensor.matmul(ps[:, osl], bqk_sb[0:1, h, :],
                                     ones_row[0:1, :], start=False, stop=True)
                hsl = slice(hg * 4, (hg + 1) * 4)
                src_q = ps[0:64, :].rearrange("p (hh t) -> p hh t", t=P)
                src_k = ps[64:128, :].rearrange("p (hh t) -> p hh t", t=P)
                eng = nc.scalar if hg % 2 == 0 else None
                if eng is not None:
                    nc.scalar.copy(q_sb[:, hsl, :], src_q)
                    nc.scalar.copy(k_sb[:, hsl, :], src_k)
                else:
                    nc.vector.tensor_copy(q_sb[:, hsl, :], src_q)
                    nc.vector.tensor_copy(k_sb[:, hsl, :], src_k)

            v_sb = vpool.tile([P, HID], f16, tag="vsb")
            for oc in range(2):
                vps = psv.tile([P, 512], f32, tag="vps")
                osl = slice(oc * 512, (oc + 1) * 512)
                for kt in range(KT):
                    nc.tensor.matmul(vps[:], xk[:, kt, :],
                                     wv_sb[:, kt, osl],
                                     start=(kt == 0), stop=False)
                nc.tensor.matmul(vps[:], ones_row[0:1, :], bv_sb[0:1, osl],
                                 start=False, stop=True)
                nc.scalar.copy(v_sb[:, osl], vps[:])

            v_scr = dpool.tile([P, H, DH], f16, tag="vscr")
            nc.sync.dma_start(v_scr[:],
                              v_sb[:].rearrange("t (g d) -> t g d", d=DH))
            vrep = vreppool.tile([P, 32, DH], f16, tag="vrep")
            vsrc = v_scr[:].rearrange("(s j4) g d -> j4 g s d", j4=4)
            for r in range(4):
                nc.sync.dma_start(vrep[32 * r: 32 * r + 16, :, :], vsrc[r])
            st[t] = (q_sb, k_sb, vrep, m2)

        def stage_b1(t):
            """Scores + exp for tile t."""
            q_sb, k_sb, vrep, m2 = st[t]
            if phase <= 1:
                o_sb = opool.tile([P, HID], f16, tag="osb")
                nc.vector.tensor_copy(o_sb[:, 0:512],
                                      vrep[:].rearrange("p s d -> p (s d)")[:, 0:512])
                nc.vector.tensor_copy(o_sb[:, 512:1024], m2[:])
                nc.sync.dma_start(out_d[t], o_sb[:])
                st[t] = (vrep, None)
                return
            for j4 in range(4):
                for slot in range(32):
                    j = slot * 4 + j4
                    nc.tensor.matmul(
                        sps[32 * j4: 32 * j4 + H, 16 * slot: 16 * slot + H],
                        k_sb[:, :, j], q_sb[:, :, j], start=True, stop=True,
                        tile_position=(0, 32 * j4))
            sm = epool.tile([P, 512], f32, tag="sm")
            nc.vector.tensor_add(sm[:], sps[:], m2[:])
            e2 = epool.tile([P, 512], f16, tag="e2")
            nc.scalar.activation(e2[:], sm[:], Act.Exp, bias=neg2[:])
            if phase <= 2:
                o_sb = opool.tile([P, HID], f16, tag="osb")
                nc.vector.tensor_copy(o_sb[:, 0:512], e2[:])
                nc.vector.tensor_copy(o_sb[:, 512:1024], sm[:])
                nc.sync.dma_start(out_d[t], o_sb[:])
                st[t] = (vrep, None)
                return
            st[t] = (vrep, e2)

        def stage_b2(t):
            """Sums, normalize, AV, output for tile t."""
            vrep, e2 = st.pop(t)
            if e2 is None:
                return
            sums = pssum.tile([P, 512], f32, tag="sums")
            nc.tensor.matmul(sums[:], obd_sb[:], e2[:], start=True, stop=True)
            r2 = epool.tile([P, 512], f32, tag="r2")
            nc.vector.reciprocal(r2[:], sums[:])
            e2s = epool.tile([P, 512], f16, tag="e2s")
            nc.vector.tensor_mul(e2s[:], e2[:], r2[:])
            if phase <= 3:
                o_sb = opool.tile([P, HID], f16, tag="osb")
                nc.vector.tensor_copy(o_sb[:, 0:512], e2s[:])
                nc.vector.tensor_copy(o_sb[:, 512:1024], r2[:])
                nc.sync.dma_start(out_d[t], o_sb[:])
                return

            avps_a = psav.tile([P, 512], f32, tag="avps_a")
            avps_b = psav.tile([P, 512], f32, tag="avps_b")
            for j4 in range(4):
                for slot in range(32):
                    j = slot * 4 + j4
                    bank = avps_a if j < 64 else avps_b
                    col = ((j // 2) % 32) * 16
                    nc.tensor.matmul(
                        bank[64 * (j % 2): 64 * (j % 2) + DH, col: col + H],
                        vrep[32 * j4: 32 * j4 + H, slot, :],
                        e2s[32 * j4: 32 * j4 + H, 16 * slot: 16 * slot + H],
                        start=True, stop=True,
                        tile_position=(32 * j4, 64 * (j % 2)))
            o_sb = opool.tile([P, HID], f16, tag="osb")
            if phase <= 3.5:
                nc.vector.tensor_copy(o_sb[:, 0:512], e2s[:])
                nc.vector.tensor_copy(o_sb[:, 512:1024], e2s[:])
            else:
                nc.scalar.copy(o_sb[:, 0:512], avps_a[:])
                nc.scalar.copy(o_sb[:, 512:1024], avps_b[:])
            nc.sync.dma_start(out_d[t], o_sb[:])

        for t in range(NT):
            stage_a(t)
            if t >= 1:
                stage_b1(t - 1)
            if t >= 2:
                stage_b2(t - 2)
        stage_b1(NT - 1)
        stage_b2(NT - 2)
        stage_b2(NT - 1)

    nc.compile()
    return nc


def _host_prep(query, W_qkv, b_qkv, attn_mask):
    scale = 1.0 / np.sqrt(DH)
    x = np.asarray(query, dtype=np.float32).reshape(T, HID)
    W = np.asarray(W_qkv, dtype=np.float32)
    b = np.asarray(b_qkv, dtype=np.float32)
    m = np.asarray(attn_mask, dtype=np.float32).reshape(T, H, H)

    # wqk[kp, h, kt, sel*64+d]
    Wq = (W[0:HID] * scale).reshape(H, DH, KT, P)      # [h, d, kt, kp]
    Wk = W[HID:2 * HID].reshape(H, DH, KT, P)
    wqk = np.stack([Wq, Wk], axis=0)                   # [sel, h, d, kt, kp]
    wqk = np.ascontiguousarray(
        wqk.transpose(4, 1, 3, 0, 2).reshape(P, H, KT, P)).astype(np.float16)
    bq = (b[0:HID] * scale).reshape(H, DH)
    bk = b[HID:2 * HID].reshape(H, DH)
    bqk = np.stack([bq, bk], axis=1).reshape(1, H, P).astype(np.float16)

    # wv[kp, kt, o]
    wv = np.ascontiguousarray(
        W[2 * HID:].reshape(HID, KT, P).transpose(2, 1, 0)).astype(np.float16)
    bv = b[2 * HID:].reshape(1, HID).astype(np.float16)

    # ones_bd: block r rows 32r..32r+15 (g), cols 32r..32r+31
    obd = np.zeros((P, P), dtype=np.float16)
    for r in range(4):
        obd[32 * r: 32 * r + H, 32 * r: 32 * r + 32] = 1.0

    # per-core xk and mask2
    xks, m2s = [], []
    for c in range(NCORES):
        xc = x[c * TC:(c + 1) * TC].reshape(NT, P, KT, P)   # [t, j, kt, kp]
        xks.append(np.ascontiguousarray(
            xc.transpose(0, 3, 2, 1)).astype(np.float16))   # [t, kp, kt, j]
        mc = m[c * TC:(c + 1) * TC].reshape(NT, 32, 4, H, H)  # [t,slot,j4,h,g]
        m2 = np.full((NT, 4, 32, 32, H), NEG, dtype=np.float32)
        m2[:, :, 0:H, :, :] = mc.transpose(0, 2, 4, 1, 3)   # [t, j4, g, slot, h]
        m2s.append(m2.reshape(NT, P, 512).astype(np.float16))
    return xks, wqk, bqk, wv, bv, m2s, obd


def kernel(query, key, value, attn_mask, W_qkv, b_qkv):
    from concourse.bass_utils import run_bass_kernel_spmd

    xks, wqk, bqk, wv, bv, m2s, obd = _host_prep(query, W_qkv, b_qkv,
                                                 attn_mask)

    if "nc" not in _compiled:
        _compiled["nc"] = _build()
    nc = _compiled["nc"]

    in_maps = []
    for c in range(NCORES):
        in_maps.append({
            "xk": xks[c], "wqk": wqk, "bqk": bqk, "wv": wv, "bv": bv,
            "m2": m2s[c], "obd": obd,
        })

    res = run_bass_kernel_spmd(nc, in_maps, core_ids=list(range(NCORES)))

    # decode parked output: arr[t, 64*(j%2)+d, 16*(j//2)+h]
    outs = []
    for c in range(NCORES):
        arr = np.asarray(res.results[c]["out"], dtype=np.float32)
        arr = arr.reshape(NT, 2, DH, 64, H)          # [t, j2, d, jh, h]
        o = arr.transpose(0, 3, 1, 4, 2).reshape(TC, HID)
        outs.append(o)
    out = np.concatenate(outs, axis=0)
    return out.reshape(B, S, HID).astype(np.float32)


if __name__ == "__main__":
    rng = np.random.default_rng(0)
    inputs = {
        "query": rng.standard_normal((B, S, HID), dtype=np.float32),
        "key": rng.standard_normal((B, S, HID), dtype=np.float32),
        "value": rng.standard_normal((B, S, HID), dtype=np.float32),
        "attn_mask": rng.standard_normal((B, S, H, H), dtype=np.float32),
        "W_qkv": (rng.standard_normal((3 * HID, HID), dtype=np.float32)
                  / np.sqrt(HID)),
        "b_qkv": rng.standard_normal((3 * HID,), dtype=np.float32) * 0.01,
    }
    out = kernel(**inputs)

    # numpy reference
    x = inputs["query"].reshape(T, HID)
    qkv = x @ inputs["W_qkv"].T + inputs["b_qkv"]
    q, k, v = np.split(qkv, 3, axis=-1)
    q = q.reshape(T, H, DH) / np.sqrt(DH)
    k = k.reshape(T, H, DH)
    v = v.reshape(T, H, DH)
    s = np.einsum("thd,tgd->thg", q, k) + inputs["attn_mask"].reshape(T, H, H)
    p = np.exp(s - s.max(-1, keepdims=True))
    p /= p.sum(-1, keepdims=True)
    o = np.einsum("thg,tgd->thd", p, v).reshape(B, S, HID)
    err = np.abs(out - o).max()
    print("kernel output:", out.shape, "abs err:", err,
          "rel:", err / np.abs(o).max())
